# revision 33
# baseline (speedup 1.0000x reference)
"""Trainium2 Bass kernel for nn_CGAMotorModel.

Reference computes, for B=512, H=1024, D=5 multivector channels of Cl(4,1):
    W_x[b,h]  = sum_d x[b,d] o W_in[h,d]          (o = geometric product)
    h_free    = (1 - (1-dt)^n) * W_x              (closed form of the scan)
    out[b]    = sum_h h_free[b,h] o W_out[h]

By associativity/bilinearity of the geometric product this collapses to
    out[b] = c * sum_d x[b,d] o K_d,   K_d = sum_h W_in[h,d] o W_out[h]
with c = 1 - 0.9^10.  On device (per core, batch-sharded 64 rows):
    S^T[r,(d,q)] = sum_h W_out[h,r] * W_in[h,(d,q)]      (8 matmuls, K=128)
    K^T[r',d]    = sum_q  C[q,:,:] slab @ S_q^T           (32 matmuls, K=32)
    M'[d,(p,m)]  = K^T.T @ CT                             (2 matmuls)
    M[(d,p),m]   = repack of M' (DRAM bounce, 2 chains)
    out[b,m]     = X^T.T @ M                              (2 matmuls, PE transposes for X^T)
where C is the Cl(4,1) Cayley table and CT[r, q*32+r'] = C[q, r, r'].
"""

import numpy as np

import concourse.bass as bass
import concourse.mybir as mybir
import concourse.tile as tile
from concourse import bacc
from concourse.bass_utils import run_bass_kernel_spmd
from concourse.masks import make_identity

B, H, D, MV = 512, 1024, 5, 32
N_CORES = 8
B_LOC = B // N_CORES
DT, N_FREE = 0.1, 10
C_SCALE = 1.0 - (1.0 - DT) ** N_FREE
F32 = mybir.dt.float32


def _cayley_np() -> np.ndarray:
    """Cayley table for Cl(4,1), metric diag(1,1,1,1,-1). C[a,b,a^b] = sign."""
    metric = np.array([1.0, 1.0, 1.0, 1.0, -1.0], dtype=np.float32)
    C = np.zeros((32, 32, 32), dtype=np.float32)
    for a in range(32):
        for b in range(32):
            cnt = 0
            aa = a >> 1
            while aa:
                cnt += bin(aa & b).count("1")
                aa >>= 1
            s = -1.0 if (cnt & 1) else 1.0
            common = a & b
            for i in range(5):
                if (common >> i) & 1:
                    s *= metric[i]
            C[a, b, a ^ b] = s
    return C


# CT[r, q*32 + r'] = C[q, r, r'] — used both as the per-q (r, r') slabs in the
# K-step and as the (r', (p, m)) right operand in the M-step.
CT = np.ascontiguousarray(_cayley_np().transpose(1, 0, 2)).reshape(32, 1024)


def build_program(
    x_eng: str = "sync",
    ct_eng: str = "sync",
    w_engs: tuple = ("gpsimd", "gpsimd"),
    rp1_eng: str = "sync",
    rp2_eng: str = "gpsimd",
    out_eng: str = "sync",
) -> bass.Bass:
    w_split = len(w_engs)
    # Bacc (not plain Bass): its compile pass moves multi-sem matmul waits
    # onto LdWeights — walrus rejects Matmult with >1 sync wait otherwise.
    nc = bacc.Bacc()
    x = nc.dram_tensor("x", [B_LOC, D * MV], F32, kind="ExternalInput")
    # wcat = [W_in.reshape(H,160) | W_out.reshape(H,32)] per H row
    wcat = nc.dram_tensor("wcat", [H, 192], F32, kind="ExternalInput")
    ct = nc.dram_tensor("ct", [32, 1024], F32, kind="ExternalInput")
    out = nc.dram_tensor("out", [B_LOC, MV], F32, kind="ExternalOutput")
    # DRAM bounce buffer for the M'[d,(p,m)] -> M[(d,p),m] repack (SBUF APs
    # can't regroup free bits into partitions; DRAM APs are flat).
    mscratch = nc.dram_tensor("mscratch", [D * MV, MV], F32)

    with tile.TileContext(nc) as tc:
        with (
            tc.tile_pool(name="sb", bufs=1) as sb,
            tc.tile_pool(name="ps", bufs=1, space="PSUM") as ps,
        ):
            # --- loads: x + constants first so PE has early work; identity
            # generated on Pool (no DMA) ---
            eng = lambda name: getattr(nc, name)
            xsb = sb.tile([B_LOC, 160], F32, tag="xsb")
            eng(x_eng).dma_start(xsb[:], x[:])
            ct_sb = sb.tile([32, 1024], F32, tag="ct_sb")
            eng(ct_eng).dma_start(ct_sb[:], ct[:])
            ident_sb = sb.tile([B_LOC, B_LOC], F32, tag="ident_sb")
            make_identity(nc, ident_sb[:])

            # --- X^T via PE transposes: (64,160) -> (128,64) + (32,64) ---
            xt1p = ps.tile([128, B_LOC], F32, tag="xt1p")
            nc.tensor.transpose(xt1p[:], xsb[:, 0:128], ident_sb[:])
            xt2p = ps.tile([32, B_LOC], F32, tag="xt2p")
            nc.tensor.transpose(xt2p[:], xsb[:, 128:160], ident_sb[:])
            xt1 = sb.tile([128, B_LOC], F32, tag="xt1")
            nc.vector.tensor_copy(xt1[:], xt1p[:])
            xt2 = sb.tile([32, B_LOC], F32, tag="xt2")
            nc.vector.tensor_copy(xt2[:], xt2p[:])

            # --- weights: w_split DMAs, each covering 8/w_split H-chunks
            # interleaved so chunk j sits at SBUF cols j*192 of its group.
            # S^T[r,(d,q)] = sum_h W_out[h,r] W_in[h,(d,q)]
            g = 8 // w_split
            w_g = []
            for s in range(w_split):
                wt = sb.tile([128, g * 192], F32, tag=f"w{s}")
                eng(w_engs[s]).dma_start(
                    wt[:].rearrange("p (t f) -> p t f", t=g),
                    wcat[128 * g * s : 128 * g * (s + 1), :].rearrange(
                        "(t p) f -> p t f", p=128
                    ),
                )
                w_g.append(wt)
            spsum = ps.tile([32, 160], F32, tag="spsum")
            for t in range(8):
                s, j = t // g, t % g
                nc.tensor.matmul(
                    spsum[:],
                    w_g[s][:, 192 * j + 160 : 192 * j + 192],
                    w_g[s][:, 192 * j : 192 * j + 160],
                    start=(t == 0),
                    stop=(t == 7),
                )
            ssb = sb.tile([32, 160], F32, tag="ssb")
            nc.vector.tensor_copy(ssb[:], spsum[:])

            # --- K-step: K^T[r',d] = sum_q C[q] slab.T @ S_q^T ---
            kpsum = ps.tile([32, D], F32, tag="kpsum")
            for q in range(32):
                nc.tensor.matmul(
                    kpsum[:],
                    ct_sb[:, 32 * q : 32 * (q + 1)],
                    ssb[:, q : 160 : 32],
                    start=(q == 0),
                    stop=(q == 31),
                )
            ksb = sb.tile([32, D], F32, tag="ksb")
            # fold the free-phase geometric-series constant in here
            nc.scalar.mul(ksb[:], kpsum[:], C_SCALE)

            # --- M-step: M'[d,(p,m)] = sum_r' K^T[r',d] * CT[r',(p,m)] ---
            m1p = ps.tile([D, 512], F32, tag="m1p")
            m2p = ps.tile([D, 512], F32, tag="m2p")
            nc.tensor.matmul(m1p[:], ksb[:], ct_sb[:, 0:512], start=True, stop=True)
            nc.tensor.matmul(m2p[:], ksb[:], ct_sb[:, 512:1024], start=True, stop=True)
            msb = sb.tile([D, 1024], F32, tag="msb")
            nc.vector.tensor_copy(msb[:, 0:512], m1p[:])
            nc.vector.tensor_copy(msb[:, 512:1024], m2p[:])

            # --- repack M'[d,(p,m)] -> M[(d,p),m] via DRAM bounce.
            # Two independent store->load chains (d<4 and d=4) on SWDGE. ---
            # Merged store + m1 load issued back-to-back on one queue (FIFO
            # ordering lets the load trail the store without a completion
            # wait); the small m2 load rides a second queue.
            m1 = sb.tile([128, 32], F32, tag="m1")
            m2 = sb.tile([32, 32], F32, tag="m2")
            eng(rp1_eng).dma_start(
                mscratch[:].rearrange("(d p) m -> d (p m)", d=D), msb[:]
            )
            eng(rp1_eng).dma_start(m1[:], mscratch[0:128, :])
            eng(rp2_eng).dma_start(m2[:], mscratch[128:160, :])

            # --- final: out[b,m] = sum_(d,p) X^T[(d,p),b] * M[(d,p),m] ---
            opsum = ps.tile([B_LOC, MV], F32, tag="opsum")
            nc.tensor.matmul(opsum[:], xt1[:], m1[:], start=True, stop=False)
            nc.tensor.matmul(opsum[:], xt2[:], m2[:], start=False, stop=True)
            osb = sb.tile([B_LOC, MV], F32, tag="osb")
            nc.vector.tensor_copy(osb[:], opsum[:])
            eng(out_eng).dma_start(out[:], osb[:])

    nc.finalize()
    return nc


_NC_CACHE: list = []


def kernel(x_mv: np.ndarray, W_in: np.ndarray, W_out: np.ndarray) -> np.ndarray:
    if not _NC_CACHE:
        _NC_CACHE.append(build_program())
    nc = _NC_CACHE[0]

    # coerce to host numpy up front — jax-array inputs would otherwise turn
    # every reshape/slice below into a device computation
    x_mv = np.asarray(x_mv)
    W_in = np.asarray(W_in)
    W_out = np.asarray(W_out)

    xf = np.ascontiguousarray(x_mv.reshape(B, D * MV).astype(np.float32))
    wcat = np.ascontiguousarray(
        np.concatenate(
            [
                W_in.reshape(H, D * MV).astype(np.float32),
                W_out.reshape(H, MV).astype(np.float32),
            ],
            axis=1,
        )
    )

    in_maps = [
        {
            "x": xf[c * B_LOC : (c + 1) * B_LOC],
            "wcat": wcat,
            "ct": CT,
        }
        for c in range(N_CORES)
    ]
    try:
        res = run_bass_kernel_spmd(nc, in_maps, core_ids=list(range(N_CORES)))
    except Exception:
        # transient NRT/device hiccups have been observed; one retry
        res = run_bass_kernel_spmd(nc, in_maps, core_ids=list(range(N_CORES)))
    out = np.concatenate([res.results[c]["out"] for c in range(N_CORES)], axis=0)
    return out.reshape(B, 1, MV)


# revision 39
# speedup vs baseline: 1.0654x; 1.0654x over previous
"""Trainium2 Bass kernel for nn_CGAMotorModel.

Reference computes, for B=512, H=1024, D=5 multivector channels of Cl(4,1):
    W_x[b,h]  = sum_d x[b,d] o W_in[h,d]          (o = geometric product)
    h_free    = (1 - (1-dt)^n) * W_x              (closed form of the scan)
    out[b]    = sum_h h_free[b,h] o W_out[h]

By associativity/bilinearity of the geometric product this collapses to
    out[b] = c * sum_d x[b,d] o K_d,   K_d = sum_h W_in[h,d] o W_out[h]
with c = 1 - 0.9^10.  On device (per core, batch-sharded 64 rows):
    S^T[r,(d,q)] = sum_h W_out[h,r] * W_in[h,(d,q)]      (8 matmuls, K=128)
    K^T[r',d]    = sum_q  C[q,:,:] slab @ S_q^T           (32 matmuls, K=32)
    M'[d,(p,m)]  = K^T.T @ CT                             (2 matmuls)
    M[(d,p),m]   = repack of M' (DRAM bounce, 2 chains)
    out[b,m]     = X^T.T @ M                              (2 matmuls, PE transposes for X^T)
where C is the Cl(4,1) Cayley table and CT[r, q*32+r'] = C[q, r, r'].
"""

import numpy as np

import concourse.bass as bass
import concourse.mybir as mybir
import concourse.tile as tile
from concourse import bacc
from concourse.bass_utils import run_bass_kernel_spmd
from concourse.masks import make_identity

B, H, D, MV = 512, 1024, 5, 32
N_CORES = 8
B_LOC = B // N_CORES
DT, N_FREE = 0.1, 10
C_SCALE = 1.0 - (1.0 - DT) ** N_FREE
F32 = mybir.dt.float32


def _cayley_np() -> np.ndarray:
    """Cayley table for Cl(4,1), metric diag(1,1,1,1,-1). C[a,b,a^b] = sign."""
    metric = np.array([1.0, 1.0, 1.0, 1.0, -1.0], dtype=np.float32)
    C = np.zeros((32, 32, 32), dtype=np.float32)
    for a in range(32):
        for b in range(32):
            cnt = 0
            aa = a >> 1
            while aa:
                cnt += bin(aa & b).count("1")
                aa >>= 1
            s = -1.0 if (cnt & 1) else 1.0
            common = a & b
            for i in range(5):
                if (common >> i) & 1:
                    s *= metric[i]
            C[a, b, a ^ b] = s
    return C


# CT[r, q*32 + r'] = C[q, r, r'] — used both as the per-q (r, r') slabs in the
# K-step and as the (r', (p, m)) right operand in the M-step.
CT = np.ascontiguousarray(_cayley_np().transpose(1, 0, 2)).reshape(32, 1024)


def build_program(
    x_eng: str = "sync",
    ct_eng: str = "gpsimd",
    w_engs: tuple = ("sync", "gpsimd", "sync", "gpsimd"),
    rp1_eng: str = "sync",
    rp2_eng: str = "gpsimd",
    out_eng: str = "sync",
    warm_dma: bool = False,
    split_store: bool = True,
) -> bass.Bass:
    w_split = len(w_engs)
    # Bacc (not plain Bass): its compile pass moves multi-sem matmul waits
    # onto LdWeights — walrus rejects Matmult with >1 sync wait otherwise.
    nc = bacc.Bacc()
    x = nc.dram_tensor("x", [B_LOC, D * MV], F32, kind="ExternalInput")
    # wcat = [W_in.reshape(H,160) | W_out.reshape(H,32)] per H row
    wcat = nc.dram_tensor("wcat", [H, 192], F32, kind="ExternalInput")
    ct = nc.dram_tensor("ct", [32, 1024], F32, kind="ExternalInput")
    out = nc.dram_tensor("out", [B_LOC, MV], F32, kind="ExternalOutput")
    # DRAM bounce buffer for the M'[d,(p,m)] -> M[(d,p),m] repack (SBUF APs
    # can't regroup free bits into partitions; DRAM APs are flat).
    mscratch = nc.dram_tensor("mscratch", [D * MV, MV], F32)

    with tile.TileContext(nc) as tc:
        with (
            tc.tile_pool(name="sb", bufs=1) as sb,
            tc.tile_pool(name="ps", bufs=1, space="PSUM") as ps,
        ):
            # --- loads: x + constants first so PE has early work; identity
            # generated on Pool (no DMA) ---
            eng = lambda name: getattr(nc, name)

            # --- weights first: they gate the critical S->K->M chain.
            # w_split DMAs, each covering 8/w_split H-chunks interleaved so
            # chunk j sits at SBUF cols j*192 of its group.
            # S^T[r,(d,q)] = sum_h W_out[h,r] W_in[h,(d,q)]
            g = 8 // w_split
            w_g = []
            for s in range(w_split):
                wt = sb.tile([128, g * 192], F32, tag=f"w{s}")
                eng(w_engs[s]).dma_start(
                    wt[:].rearrange("p (t f) -> p t f", t=g),
                    wcat[128 * g * s : 128 * g * (s + 1), :].rearrange(
                        "(t p) f -> p t f", p=128
                    ),
                )
                w_g.append(wt)

            ct_sb = sb.tile([32, 1024], F32, tag="ct_sb")
            eng(ct_eng).dma_start(ct_sb[:], ct[:])
            xsb = sb.tile([B_LOC, 160], F32, tag="xsb")
            eng(x_eng).dma_start(xsb[:], x[:])
            ident_sb = sb.tile([B_LOC, B_LOC], F32, tag="ident_sb")
            make_identity(nc, ident_sb[:])
            spsum = ps.tile([32, 160], F32, tag="spsum")
            for t in range(8):
                s, j = t // g, t % g
                nc.tensor.matmul(
                    spsum[:],
                    w_g[s][:, 192 * j + 160 : 192 * j + 192],
                    w_g[s][:, 192 * j : 192 * j + 160],
                    start=(t == 0),
                    stop=(t == 7),
                )
            ssb = sb.tile([32, 160], F32, tag="ssb")
            nc.vector.tensor_copy(ssb[:], spsum[:])

            # --- K-step: K^T[r',d] = sum_q C[q] slab.T @ S_q^T ---
            kpsum = ps.tile([32, D], F32, tag="kpsum")
            for q in range(32):
                nc.tensor.matmul(
                    kpsum[:],
                    ct_sb[:, 32 * q : 32 * (q + 1)],
                    ssb[:, q : 160 : 32],
                    start=(q == 0),
                    stop=(q == 31),
                )
            ksb = sb.tile([32, D], F32, tag="ksb")
            # fold the free-phase geometric-series constant in here
            nc.scalar.mul(ksb[:], kpsum[:], C_SCALE)

            # --- X^T via PE transposes, slotted into the PE bubble while the
            # K->ksb->M' semaphore round-trip is in flight ---
            xt1p = ps.tile([128, B_LOC], F32, tag="xt1p")
            nc.tensor.transpose(xt1p[:], xsb[:, 0:128], ident_sb[:])
            xt2p = ps.tile([32, B_LOC], F32, tag="xt2p")
            nc.tensor.transpose(xt2p[:], xsb[:, 128:160], ident_sb[:])
            xt1 = sb.tile([128, B_LOC], F32, tag="xt1")
            nc.vector.tensor_copy(xt1[:], xt1p[:])
            xt2 = sb.tile([32, B_LOC], F32, tag="xt2")
            nc.vector.tensor_copy(xt2[:], xt2p[:])

            # --- M-step: M'[d,(p,m)] = sum_r' K^T[r',d] * CT[r',(p,m)] ---
            m1p = ps.tile([D, 512], F32, tag="m1p")
            m2p = ps.tile([D, 512], F32, tag="m2p")
            nc.tensor.matmul(m1p[:], ksb[:], ct_sb[:, 0:512], start=True, stop=True)
            nc.tensor.matmul(m2p[:], ksb[:], ct_sb[:, 512:1024], start=True, stop=True)
            msb = sb.tile([D, 1024], F32, tag="msb")
            nc.vector.tensor_copy(msb[:, 0:512], m1p[:])
            nc.vector.tensor_copy(msb[:, 512:1024], m2p[:])

            # --- repack M'[d,(p,m)] -> M[(d,p),m] via DRAM bounce.
            # Two independent store->load chains (d<4 and d=4) on SWDGE. ---
            # Merged store + m1 load issued back-to-back on one queue (FIFO
            # ordering lets the load trail the store without a completion
            # wait); the small m2 load rides a second queue.
            m1 = sb.tile([128, 32], F32, tag="m1")
            m2 = sb.tile([32, 32], F32, tag="m2")
            if warm_dma:
                warm = sb.tile([1, 32], F32, tag="warm")
                eng(rp1_eng).dma_start(warm[:], ct[0:1, 0:32])
            if split_store:
                msc = mscratch[:].rearrange("(d p) m -> d p m", p=32)
                eng(rp1_eng).dma_start(
                    msc[:, 0:16, :],
                    msb[:, 0:512].rearrange("d (p m) -> d p m", m=32),
                )
                eng(rp1_eng).dma_start(
                    msc[:, 16:32, :],
                    msb[:, 512:1024].rearrange("d (p m) -> d p m", m=32),
                )
            else:
                eng(rp1_eng).dma_start(
                    mscratch[:].rearrange("(d p) m -> d (p m)", d=D), msb[:]
                )
            eng(rp1_eng).dma_start(m1[:], mscratch[0:128, :])
            eng(rp2_eng).dma_start(m2[:], mscratch[128:160, :])

            # --- final: out[b,m] = sum_(d,p) X^T[(d,p),b] * M[(d,p),m] ---
            opsum = ps.tile([B_LOC, MV], F32, tag="opsum")
            nc.tensor.matmul(opsum[:], xt1[:], m1[:], start=True, stop=False)
            nc.tensor.matmul(opsum[:], xt2[:], m2[:], start=False, stop=True)
            osb = sb.tile([B_LOC, MV], F32, tag="osb")
            nc.vector.tensor_copy(osb[:], opsum[:])
            eng(out_eng).dma_start(out[:], osb[:])

    nc.finalize()
    return nc


_NC_CACHE: list = []


def kernel(x_mv: np.ndarray, W_in: np.ndarray, W_out: np.ndarray) -> np.ndarray:
    if not _NC_CACHE:
        _NC_CACHE.append(build_program())
    nc = _NC_CACHE[0]

    # coerce to host numpy up front — jax-array inputs would otherwise turn
    # every reshape/slice below into a device computation
    x_mv = np.asarray(x_mv)
    W_in = np.asarray(W_in)
    W_out = np.asarray(W_out)

    xf = np.ascontiguousarray(x_mv.reshape(B, D * MV).astype(np.float32))
    wcat = np.ascontiguousarray(
        np.concatenate(
            [
                W_in.reshape(H, D * MV).astype(np.float32),
                W_out.reshape(H, MV).astype(np.float32),
            ],
            axis=1,
        )
    )

    in_maps = [
        {
            "x": xf[c * B_LOC : (c + 1) * B_LOC],
            "wcat": wcat,
            "ct": CT,
        }
        for c in range(N_CORES)
    ]
    try:
        res = run_bass_kernel_spmd(nc, in_maps, core_ids=list(range(N_CORES)))
    except Exception:
        # transient NRT/device hiccups have been observed; one retry
        res = run_bass_kernel_spmd(nc, in_maps, core_ids=list(range(N_CORES)))
    out = np.concatenate([res.results[c]["out"] for c in range(N_CORES)], axis=0)
    return out.reshape(B, 1, MV)


# revision 44
# speedup vs baseline: 1.0679x; 1.0024x over previous
"""Trainium2 Bass kernel for nn_CGAMotorModel.

Reference computes, for B=512, H=1024, D=5 multivector channels of Cl(4,1):
    W_x[b,h]  = sum_d x[b,d] o W_in[h,d]          (o = geometric product)
    h_free    = (1 - (1-dt)^n) * W_x              (closed form of the scan)
    out[b]    = sum_h h_free[b,h] o W_out[h]

By associativity/bilinearity of the geometric product this collapses to
    out[b] = c * sum_d x[b,d] o K_d,   K_d = sum_h W_in[h,d] o W_out[h]
with c = 1 - 0.9^10.  On device (per core, batch-sharded 64 rows):
    S^T[r,(d,q)] = sum_h W_out[h,r] * W_in[h,(d,q)]      (8 matmuls, K=128)
    K^T[r',d]    = sum_q  C[q,:,:] slab @ S_q^T           (32 matmuls, K=32)
    M'[d,(p,m)]  = K^T.T @ CT                             (2 matmuls)
    M[(d,p),m]   = repack of M' (DRAM bounce, 2 chains)
    out[b,m]     = X^T.T @ M                              (2 matmuls, PE transposes for X^T)
where C is the Cl(4,1) Cayley table and CT[r, q*32+r'] = C[q, r, r'].
"""

import numpy as np

import concourse.bass as bass
import concourse.mybir as mybir
import concourse.tile as tile
from concourse import bacc
from concourse.bass_utils import run_bass_kernel_spmd
from concourse.masks import make_identity

B, H, D, MV = 512, 1024, 5, 32
N_CORES = 8
B_LOC = B // N_CORES
DT, N_FREE = 0.1, 10
C_SCALE = 1.0 - (1.0 - DT) ** N_FREE
F32 = mybir.dt.float32


def _cayley_np() -> np.ndarray:
    """Cayley table for Cl(4,1), metric diag(1,1,1,1,-1). C[a,b,a^b] = sign."""
    metric = np.array([1.0, 1.0, 1.0, 1.0, -1.0], dtype=np.float32)
    C = np.zeros((32, 32, 32), dtype=np.float32)
    for a in range(32):
        for b in range(32):
            cnt = 0
            aa = a >> 1
            while aa:
                cnt += bin(aa & b).count("1")
                aa >>= 1
            s = -1.0 if (cnt & 1) else 1.0
            common = a & b
            for i in range(5):
                if (common >> i) & 1:
                    s *= metric[i]
            C[a, b, a ^ b] = s
    return C


# CT[r, q*32 + r'] = C[q, r, r'] — used both as the per-q (r, r') slabs in the
# K-step and as the (r', (p, m)) right operand in the M-step. CTK carries the
# free-phase geometric-series constant so the K->M' PSUM copy is a plain DVE
# copy (DVE->PE sem link is far cheaper than ACT->PE).
CT = np.ascontiguousarray(_cayley_np().transpose(1, 0, 2)).reshape(32, 1024)
CTK = (C_SCALE * CT).astype(np.float32)


def build_program(
    x_eng: str = "sync",
    ct_eng: str = "gpsimd",
    w_engs: tuple = ("sync", "gpsimd", "sync", "gpsimd"),
    rp1_eng: str = "sync",
    rp2_eng: str = "gpsimd",
    out_eng: str = "sync",
    warm_dma: bool = False,
    split_store: bool = True,
) -> bass.Bass:
    w_split = len(w_engs)
    # Bacc (not plain Bass): its compile pass moves multi-sem matmul waits
    # onto LdWeights — walrus rejects Matmult with >1 sync wait otherwise.
    nc = bacc.Bacc()
    x = nc.dram_tensor("x", [B_LOC, D * MV], F32, kind="ExternalInput")
    # wcat = [W_in.reshape(H,160) | W_out.reshape(H,32)] per H row
    wcat = nc.dram_tensor("wcat", [H, 192], F32, kind="ExternalInput")
    ct = nc.dram_tensor("ct", [32, 1024], F32, kind="ExternalInput")
    ctk = nc.dram_tensor("ctk", [32, 1024], F32, kind="ExternalInput")
    out = nc.dram_tensor("out", [B_LOC, MV], F32, kind="ExternalOutput")
    # DRAM bounce buffer for the M'[d,(p,m)] -> M[(d,p),m] repack (SBUF APs
    # can't regroup free bits into partitions; DRAM APs are flat).
    mscratch = nc.dram_tensor("mscratch", [D * MV, MV], F32)

    with tile.TileContext(nc) as tc:
        with (
            tc.tile_pool(name="sb", bufs=1) as sb,
            tc.tile_pool(name="ps", bufs=1, space="PSUM") as ps,
        ):
            # --- loads: x + constants first so PE has early work; identity
            # generated on Pool (no DMA) ---
            eng = lambda name: getattr(nc, name)

            # --- weights first: they gate the critical S->K->M chain.
            # w_split DMAs, each covering 8/w_split H-chunks interleaved so
            # chunk j sits at SBUF cols j*192 of its group.
            # S^T[r,(d,q)] = sum_h W_out[h,r] W_in[h,(d,q)]
            g = 8 // w_split
            w_g = []
            for s in range(w_split):
                wt = sb.tile([128, g * 192], F32, tag=f"w{s}")
                eng(w_engs[s]).dma_start(
                    wt[:].rearrange("p (t f) -> p t f", t=g),
                    wcat[128 * g * s : 128 * g * (s + 1), :].rearrange(
                        "(t p) f -> p t f", p=128
                    ),
                )
                w_g.append(wt)

            ct_sb = sb.tile([32, 1024], F32, tag="ct_sb")
            eng(ct_eng).dma_start(ct_sb[:], ct[:])
            ctk_sb = sb.tile([32, 1024], F32, tag="ctk_sb")
            eng(ct_eng).dma_start(ctk_sb[:], ctk[:])
            xsb = sb.tile([B_LOC, 160], F32, tag="xsb")
            eng(x_eng).dma_start(xsb[:], x[:])
            ident_sb = sb.tile([B_LOC, B_LOC], F32, tag="ident_sb")
            make_identity(nc, ident_sb[:])
            spsum = ps.tile([32, 160], F32, tag="spsum")
            for t in range(8):
                s, j = t // g, t % g
                nc.tensor.matmul(
                    spsum[:],
                    w_g[s][:, 192 * j + 160 : 192 * j + 192],
                    w_g[s][:, 192 * j : 192 * j + 160],
                    start=(t == 0),
                    stop=(t == 7),
                )
            ssb = sb.tile([32, 160], F32, tag="ssb")
            nc.vector.tensor_copy(ssb[:], spsum[:])

            # --- K-step: K^T[r',d] = sum_q C[q] slab.T @ S_q^T ---
            kpsum = ps.tile([32, D], F32, tag="kpsum")
            for q in range(32):
                nc.tensor.matmul(
                    kpsum[:],
                    ctk_sb[:, 32 * q : 32 * (q + 1)],
                    ssb[:, q : 160 : 32],
                    start=(q == 0),
                    stop=(q == 31),
                )
            ksb = sb.tile([32, D], F32, tag="ksb")
            nc.vector.tensor_copy(ksb[:], kpsum[:])

            # --- X^T via PE transposes, slotted into the PE bubble while the
            # K->ksb->M' semaphore round-trip is in flight ---
            xt1p = ps.tile([128, B_LOC], F32, tag="xt1p")
            nc.tensor.transpose(xt1p[:], xsb[:, 0:128], ident_sb[:])
            xt2p = ps.tile([32, B_LOC], F32, tag="xt2p")
            nc.tensor.transpose(xt2p[:], xsb[:, 128:160], ident_sb[:])
            xt1 = sb.tile([128, B_LOC], F32, tag="xt1")
            nc.vector.tensor_copy(xt1[:], xt1p[:])
            xt2 = sb.tile([32, B_LOC], F32, tag="xt2")
            nc.vector.tensor_copy(xt2[:], xt2p[:])

            # --- M-step: M'[d,(p,m)] = sum_r' K^T[r',d] * CT[r',(p,m)] ---
            m1p = ps.tile([D, 512], F32, tag="m1p")
            m2p = ps.tile([D, 512], F32, tag="m2p")
            nc.tensor.matmul(m1p[:], ksb[:], ct_sb[:, 0:512], start=True, stop=True)
            nc.tensor.matmul(m2p[:], ksb[:], ct_sb[:, 512:1024], start=True, stop=True)
            msb = sb.tile([D, 1024], F32, tag="msb")
            nc.vector.tensor_copy(msb[:, 0:512], m1p[:])
            nc.vector.tensor_copy(msb[:, 512:1024], m2p[:])

            # --- repack M'[d,(p,m)] -> M[(d,p),m] via DRAM bounce.
            # Two independent store->load chains (d<4 and d=4) on SWDGE. ---
            # Merged store + m1 load issued back-to-back on one queue (FIFO
            # ordering lets the load trail the store without a completion
            # wait); the small m2 load rides a second queue.
            m1 = sb.tile([128, 32], F32, tag="m1")
            m2 = sb.tile([32, 32], F32, tag="m2")
            if warm_dma:
                warm = sb.tile([1, 32], F32, tag="warm")
                eng(rp1_eng).dma_start(warm[:], ct[0:1, 0:32])
            if split_store:
                msc = mscratch[:].rearrange("(d p) m -> d p m", p=32)
                eng(rp1_eng).dma_start(
                    msc[:, 0:16, :],
                    msb[:, 0:512].rearrange("d (p m) -> d p m", m=32),
                )
                eng(rp1_eng).dma_start(
                    msc[:, 16:32, :],
                    msb[:, 512:1024].rearrange("d (p m) -> d p m", m=32),
                )
            else:
                eng(rp1_eng).dma_start(
                    mscratch[:].rearrange("(d p) m -> d (p m)", d=D), msb[:]
                )
            eng(rp1_eng).dma_start(m1[:], mscratch[0:128, :])
            eng(rp2_eng).dma_start(m2[:], mscratch[128:160, :])

            # --- final: out[b,m] = sum_(d,p) X^T[(d,p),b] * M[(d,p),m] ---
            opsum = ps.tile([B_LOC, MV], F32, tag="opsum")
            nc.tensor.matmul(opsum[:], xt1[:], m1[:], start=True, stop=False)
            nc.tensor.matmul(opsum[:], xt2[:], m2[:], start=False, stop=True)
            osb = sb.tile([B_LOC, MV], F32, tag="osb")
            nc.vector.tensor_copy(osb[:], opsum[:])
            eng(out_eng).dma_start(out[:], osb[:])

    nc.finalize()
    return nc


_NC_CACHE: list = []


def kernel(x_mv: np.ndarray, W_in: np.ndarray, W_out: np.ndarray) -> np.ndarray:
    if not _NC_CACHE:
        _NC_CACHE.append(build_program())
    nc = _NC_CACHE[0]

    # coerce to host numpy up front — jax-array inputs would otherwise turn
    # every reshape/slice below into a device computation
    x_mv = np.asarray(x_mv)
    W_in = np.asarray(W_in)
    W_out = np.asarray(W_out)

    xf = np.ascontiguousarray(x_mv.reshape(B, D * MV).astype(np.float32))
    wcat = np.ascontiguousarray(
        np.concatenate(
            [
                W_in.reshape(H, D * MV).astype(np.float32),
                W_out.reshape(H, MV).astype(np.float32),
            ],
            axis=1,
        )
    )

    in_maps = [
        {
            "x": xf[c * B_LOC : (c + 1) * B_LOC],
            "wcat": wcat,
            "ct": CT,
            "ctk": CTK,
        }
        for c in range(N_CORES)
    ]
    try:
        res = run_bass_kernel_spmd(nc, in_maps, core_ids=list(range(N_CORES)))
    except Exception:
        # transient NRT/device hiccups have been observed; one retry
        res = run_bass_kernel_spmd(nc, in_maps, core_ids=list(range(N_CORES)))
    out = np.concatenate([res.results[c]["out"] for c in range(N_CORES)], axis=0)
    return out.reshape(B, 1, MV)
